# revision 1
# baseline (speedup 1.0000x reference)
"""Trainium2 Bass kernel for FlattenSELayer (segment mean -> SE MLP -> gather
multiply), data-parallel over 8 NeuronCores.

Per core (rows sharded across cores):
  pass 1: segment-sum via PE matmuls with bf16 x sub-tiles stationary and a
          per-row one-hot(idx) as the moving operand; counts accumulated on
          DVE. AllReduce of the tiny (129,16) partial over the 8 cores.
          (bf16 is ample here: pooled means are O(1/sqrt(n)) and the sigmoid
          gate sits near 0.5, so segment-sum rounding is damped to ~1e-5 in
          the final output.)
  epilogue: pooled = seg_sum/counts, SE MLP (relu/sigmoid) -> gate (16,128).
  pass 2: gather gate rows back to points via one-hotT matmuls (gate split
          into bf16 hi+lo for near-f32 accuracy), multiply with f32 x, store.

Traffic per core ~161 MB (32 bf16 read + 64 f32 read + 64 f32 write).
Chunk layout "(p t) c" keeps big DMAs in 8-16 KiB per-partition runs;
pass-1 idx arrives pre-permuted from the host as one contiguous load.
"""
import sys
import types

import numpy as np

# ── shim the missing antenv.axon_hooks so run_bass_kernel_spmd imports ──
if "antenv.axon_hooks" not in sys.modules:
    _hooks = types.ModuleType("antenv.axon_hooks")
    _hooks._hook = None
    _hooks.set_axon_ntff_profile_hook = lambda h: setattr(_hooks, "_hook", h)
    _hooks.get_axon_ntff_profile_hook = lambda: _hooks._hook
    sys.modules["antenv.axon_hooks"] = _hooks
    import antenv

    antenv.axon_hooks = _hooks

import concourse.bass as bass
import concourse.bacc as bacc
import concourse.tile as tile
import concourse.mybir as mybir
from concourse.bass_utils import run_bass_kernel_spmd

F32 = mybir.dt.float32
BF16 = mybir.dt.bfloat16
FP8 = mybir.dt.float8e4
NP_BF16 = mybir.dt.np(BF16)
NP_FP8 = mybir.dt.np(FP8)

N_CORES = 8
P = 128          # partitions / rows per sub-tile
C = 128          # channels
S = 16           # num segments
HID = 32         # SE hidden dim
T_CHUNK = 32     # sub-tiles per chunk (4096 rows)
T_HALF = 16      # sub-tiles per PSUM gather tile

N_FULL = 1_000_000
SUBTILES = (N_FULL + N_CORES * P - 1) // (N_CORES * P)   # 977
ROWS_PER_CORE = SUBTILES * P                             # 125056
N_PAD = ROWS_PER_CORE * N_CORES                          # 1000448


def _chunks(subtiles, t_chunk):
    out = []
    done = 0
    while done < subtiles:
        t = min(t_chunk, subtiles - done)
        out.append((done * P, t))
        done += t
    return out


def _halves(tu):
    out = []
    done = 0
    while done < tu:
        t = min(T_HALF, tu - done)
        out.append((done, t))
        done += t
    return out


T1_CHUNK = 64


def build_kernel(rows_per_core=ROWS_PER_CORE, t_chunk=T_CHUNK):
    assert rows_per_core % P == 0
    subtiles = rows_per_core // P
    chunks = _chunks(subtiles, t_chunk)
    chunks1 = _chunks(subtiles, T1_CHUNK)

    nc = bacc.Bacc("TRN2", target_bir_lowering=False, debug=False,
                   num_devices=N_CORES)

    # x twice: bf16 for pass-1 segment sums, f32 for pass 2's multiply
    xh_in = nc.dram_tensor("xh", [rows_per_core, C], FP8,
                           kind="ExternalInput")
    x_in = nc.dram_tensor("x", [rows_per_core, C], F32, kind="ExternalInput")
    idx_in = nc.dram_tensor("idxf", [rows_per_core], F32,
                            kind="ExternalInput")
    idx8_in = nc.dram_tensor("idx8", [rows_per_core], FP8,
                             kind="ExternalInput")
    # pass-1 per-partition idx, host-permuted: [128, subtiles] where column
    # block u holds idx[base_u + p*tu + t]
    idxp_in = nc.dram_tensor("idxp", [P, subtiles], FP8,
                             kind="ExternalInput")
    w1t_in = nc.dram_tensor("w1t", [C, HID], F32, kind="ExternalInput")
    w2t_in = nc.dram_tensor("w2t", [HID, C], F32, kind="ExternalInput")
    iota_row_in = nc.dram_tensor("iota_row", [P, S], F32,
                                 kind="ExternalInput")
    iota_col_in = nc.dram_tensor("iota_col", [P, 1], F32,
                                 kind="ExternalInput")
    out_t = nc.dram_tensor("out", [rows_per_core, C], F32,
                           kind="ExternalOutput")

    xh_ap = xh_in.ap()
    x_ap = x_in.ap()
    idx_ap = idx_in.ap()
    idx8_ap = idx8_in.ap()
    out_ap = out_t.ap()

    with tile.TileContext(nc) as tc:
        with (
            tc.tile_pool(name="cst", bufs=1) as cst,
            tc.tile_pool(name="xp1", bufs=2) as xp1,
            tc.tile_pool(name="oh1", bufs=3) as oh1,
            tc.tile_pool(name="xp2", bufs=7) as xp2,
            tc.tile_pool(name="ib2", bufs=2) as ib2,
            tc.tile_pool(name="oh2", bufs=2) as oh2,
            tc.tile_pool(name="op2", bufs=4) as op2,
            tc.tile_pool(name="dram", bufs=1, space="DRAM") as dram,
        ):
            # constants
            iota_row = cst.tile([P, S], F32)
            nc.sync.dma_start(out=iota_row[:], in_=iota_row_in.ap())
            iota_col = cst.tile([P, 1], F32)
            nc.sync.dma_start(out=iota_col[:], in_=iota_col_in.ap())
            w1t_sb = cst.tile([C, HID], F32)
            nc.sync.dma_start(out=w1t_sb[:], in_=w1t_in.ap())
            w2t_sb = cst.tile([HID, C], F32)
            nc.sync.dma_start(out=w2t_sb[:], in_=w2t_in.ap())
            ones128 = cst.tile([P, 1], FP8)
            nc.vector.memset(ones128[:], 1.0)
            ones_row = cst.tile([1, P], F32)
            nc.vector.memset(ones_row[:], 1.0)
            idx_p1 = cst.tile([P, subtiles], FP8)
            nc.gpsimd.dma_start(out=idx_p1[:], in_=idxp_in.ap())

            # ───────────────────────── pass 1 ─────────────────────────
            with tc.tile_pool(name="ps1", bufs=1, space="PSUM") as ps1:
                psum_seg = ps1.tile([C, S], F32)
                psum_cnt = ps1.tile([1, T_CHUNK * S], F32)

                n_chunk = 0
                n_sub_done = 0
                sub_off = 0
                for base, tu in chunks1:
                    rows = tu * P
                    x_t = xp1.tile([P, tu, C], FP8, tag="x1", name="x1")
                    nc.sync.dma_start(
                        out=x_t[:],
                        in_=xh_ap[base:base + rows].rearrange(
                            "(p t) c -> p t c", p=P, t=tu),
                    )
                    idx_t = idx_p1[:, sub_off:sub_off + tu]
                    sub_off += tu
                    oh_t = oh1.tile([P, tu, S], FP8, tag="oh1", name="oh1")
                    idx_b = bass.AP(tensor=idx_t.tensor,
                                    offset=idx_t.offset,
                                    ap=[idx_t.ap[0], idx_t.ap[1], [0, S]])
                    iota_b = bass.AP(tensor=iota_row[:].tensor,
                                     offset=iota_row[:].offset,
                                     ap=[iota_row[:].ap[0], [0, tu],
                                         iota_row[:].ap[1]])
                    nc.vector.tensor_tensor(oh_t[:], idx_b, iota_b,
                                            mybir.AluOpType.is_equal)
                    n_chunk += 1
                    last_chunk = n_chunk == len(chunks1)
                    cnt_halves = _halves(tu)
                    for ci, (c0, ct) in enumerate(cnt_halves):
                        nc.tensor.matmul(
                            psum_cnt[:, 0:ct * S],
                            ones128[:],
                            oh_t[:, c0:c0 + ct, :].rearrange(
                                "p t s -> p (t s)"),
                            start=(n_chunk == 1 and ci == 0),
                            stop=(last_chunk and ci == len(cnt_halves) - 1),
                        )
                    for t in range(tu):
                        n_sub_done += 1
                        nc.tensor.matmul(
                            psum_seg[:],
                            x_t[:, t, :],
                            oh_t[:, t, :],
                            start=(n_sub_done == 1),
                            stop=(n_sub_done == subtiles),
                        )

                # ─────────────────── epilogue / MLP ───────────────────
                seg_sb = cst.tile([C, S], F32)
                nc.vector.tensor_copy(seg_sb[:], psum_seg[:])
                cnt_sb = cst.tile([1, T_CHUNK * S], F32)
                nc.vector.tensor_copy(cnt_sb[:], psum_cnt[:])
                w = T_CHUNK * S
                while w > S:
                    w //= 2
                    nc.vector.tensor_tensor(cnt_sb[:, 0:w], cnt_sb[:, 0:w],
                                            cnt_sb[:, w:2 * w],
                                            mybir.AluOpType.add)
                cnt16 = cnt_sb[:, 0:S]

                bounce_in = dram.tile([P + 1, S], F32)
                nc.sync.dma_start(out=bounce_in[0:C, :], in_=seg_sb[:])
                nc.sync.dma_start(out=bounce_in[C:C + 1, :], in_=cnt16)
                bounce_out = dram.tile([N_CORES, P + 1, S], F32,
                                       addr_space="Shared")
                nc.gpsimd.collective_compute(
                    "AllGather",
                    mybir.AluOpType.bypass,
                    replica_groups=[list(range(N_CORES))],
                    ins=[bounce_in[:].opt()],
                    outs=[bounce_out[:].opt()],
                )
                bo = bounce_out[:]
                seg_r = cst.tile([C, N_CORES, S], F32)
                nc.sync.dma_start(
                    out=seg_r[:],
                    in_=bass.AP(tensor=bo.tensor, offset=bo.offset,
                                ap=[[S, C], [(P + 1) * S, N_CORES],
                                    [1, S]]),
                )
                cnt_r = cst.tile([1, N_CORES, S], F32)
                nc.sync.dma_start(
                    out=cnt_r[:],
                    in_=bass.AP(tensor=bo.tensor,
                                offset=bo.offset + C * S,
                                ap=[[0, 1], [(P + 1) * S, N_CORES],
                                    [1, S]]),
                )
                w = N_CORES
                while w > 1:
                    w //= 2
                    nc.vector.tensor_tensor(
                        seg_r[:, 0:w, :], seg_r[:, 0:w, :],
                        seg_r[:, w:2 * w, :], mybir.AluOpType.add)
                    nc.vector.tensor_tensor(
                        cnt_r[:, 0:w, :], cnt_r[:, 0:w, :],
                        cnt_r[:, w:2 * w, :], mybir.AluOpType.add)
                seg_g = seg_r[:, 0, :]
                cnt_g = cnt_r[:, 0, :]

                nc.vector.tensor_scalar(cnt_g, cnt_g, 1.0, None,
                                        mybir.AluOpType.max)
                rcnt = cst.tile([1, S], F32)
                nc.vector.reciprocal(rcnt[:], cnt_g)
                rcnt_psum = ps1.tile([C, S], F32)
                nc.tensor.matmul(rcnt_psum[:], ones_row[:], rcnt[:],
                                 start=True, stop=True)
                pooledT = cst.tile([C, S], F32)
                nc.vector.tensor_tensor(pooledT[:], seg_g, rcnt_psum[:],
                                        mybir.AluOpType.mult)

                h_psum = ps1.tile([HID, S], F32)
                nc.tensor.matmul(h_psum[:], w1t_sb[:], pooledT[:],
                                 start=True, stop=True)
                hT_sb = cst.tile([HID, S], F32)
                nc.scalar.activation(hT_sb[:], h_psum[:],
                                     mybir.ActivationFunctionType.Relu)
                g_psum = ps1.tile([S, C], F32)
                nc.tensor.matmul(g_psum[:], hT_sb[:], w2t_sb[:],
                                 start=True, stop=True)
                gate_sb = cst.tile([S, C], F32)
                nc.scalar.activation(gate_sb[:], g_psum[:],
                                     mybir.ActivationFunctionType.Sigmoid)
                # split gate into bf16 hi + lo so the gather matmuls run at
                # bf16 speed with ~f32 accuracy (PSUM accumulates in f32)
                g_hi4 = cst.tile([P, C], BF16)
                nc.vector.tensor_copy(g_hi4[0:S, :], gate_sb[:])
                g_lo4 = cst.tile([P, C], BF16)
                nc.vector.tensor_tensor(g_lo4[0:S, :], gate_sb[:],
                                        g_hi4[0:S, :],
                                        mybir.AluOpType.subtract)
                for q in range(1, 3):
                    nc.sync.dma_start(out=g_hi4[32 * q:32 * q + S, :],
                                      in_=g_hi4[0:S, :])
                    nc.sync.dma_start(out=g_lo4[32 * q:32 * q + S, :],
                                      in_=g_lo4[0:S, :])

            # ───────────────────────── pass 2 ─────────────────────────
            # group up to 4 full chunks at 32-partition alignment: one
            # stacked idx broadcast-gather + one is_equal builds all their
            # one-hotT tiles (PE weight tiles may sit at partition 0/32/64/96)
            groups = []
            gi = 0
            while gi < len(chunks):
                g = [chunks[gi]]
                gi += 1
                while (gi < len(chunks) and len(g) < 3
                       and chunks[gi][1] == g[0][1]):
                    g.append(chunks[gi])
                    gi += 1
                groups.append(g)
            # put the irregular remainder group first so the kernel tail
            # stays in pipelined steady-state
            groups = groups[-1:] + groups[:-1]

            with tc.tile_pool(name="ps2", bufs=2, space="PSUM") as ps2:
                for grp in groups:
                    ng = len(grp)
                    tu = grp[0][1]
                    rows = tu * P
                    gbase = grp[0][0]
                    # stacked idx: partition 16*g+s reads chunk g's idx row
                    idxs_t = ib2.tile([32 * ng, tu * P], FP8, tag="ib2",
                                      name="ib2")
                    src_ap = idx8_ap[gbase:gbase + ng * rows]
                    nc.gpsimd.dma_start(
                        out=idxs_t[:],
                        in_=bass.AP(tensor=src_ap.tensor,
                                    offset=src_ap.offset,
                                    ap=[[rows, ng], [0, 32], [1, rows]]),
                    )
                    ohT_t = oh2.tile([32 * ng, P, tu], BF16, tag="oh2",
                                     name="ohT")
                    nc.vector.tensor_scalar(
                        ohT_t[:].rearrange("s p t -> s (p t)"),
                        idxs_t[:], iota_col[0:32 * ng, :], None,
                        mybir.AluOpType.is_equal)
                    for g, (base, _tu) in enumerate(grp):
                        x2_t = xp2.tile([P, tu, C], F32, tag="x2",
                                        name="x2")
                        nc.sync.dma_start(
                            out=x2_t[:],
                            in_=x_ap[base:base + rows].rearrange(
                                "(p t) c -> p t c", p=P, t=tu),
                        )
                        for h0, th in _halves(tu):
                            o_t = op2.tile([P, T_HALF, C], F32, tag="o2",
                                           name="o2")
                            gath = ps2.tile([P, T_HALF, C], F32,
                                            tag="gath", name="gath")
                            for t in range(h0, h0 + th):
                                nc.tensor.matmul(
                                    gath[:, t - h0, :],
                                    ohT_t[32 * g:32 * g + S, :, t],
                                    g_hi4[32 * g:32 * g + S, :],
                                    start=True, stop=False,
                                )
                                nc.tensor.matmul(
                                    gath[:, t - h0, :],
                                    ohT_t[32 * g:32 * g + S, :, t],
                                    g_lo4[32 * g:32 * g + S, :],
                                    start=False, stop=True,
                                )
                            nc.vector.tensor_tensor(
                                o_t[:, 0:th, :].rearrange(
                                    "p t c -> p (t c)"),
                                x2_t[:, h0:h0 + th, :].rearrange(
                                    "p t c -> p (t c)"),
                                gath[:, 0:th, :].rearrange(
                                    "p t c -> p (t c)"),
                                mybir.AluOpType.mult,
                            )
                            nc.scalar.dma_start(
                                out=bass.AP(
                                    tensor=out_ap.tensor,
                                    offset=out_ap.offset
                                    + (base + h0) * C,
                                    ap=[[tu * C, P], [C, th], [1, C]]),
                                in_=o_t[:, 0:th, :],
                            )

    nc.compile()
    return nc


_NC_CACHE = {}


def _get_nc(rows_per_core=ROWS_PER_CORE, t_chunk=T_CHUNK):
    key = (rows_per_core, t_chunk)
    if key not in _NC_CACHE:
        _NC_CACHE[key] = build_kernel(rows_per_core, t_chunk)
    return _NC_CACHE[key]


def _permute_idx_p1(idx_core, subtiles, t_chunk):
    """[rows] -> [128, subtiles]; block u holds idx[base_u + p*tu + t]."""
    cols = []
    for base, tu in _chunks(subtiles, 64):
        cols.append(idx_core[base:base + tu * P].reshape(P, tu))
    return np.concatenate(cols, axis=1)


def make_in_maps(x, indices, W1, W2, rows_per_core=ROWS_PER_CORE,
                 t_chunk=T_CHUNK):
    n = x.shape[0]
    subtiles = rows_per_core // P
    n_pad = rows_per_core * N_CORES
    xp = np.zeros((n_pad, C), dtype=np.float32)
    xp[:n] = np.asarray(x, dtype=np.float32)
    xh = xp.astype(NP_FP8)
    idxp = np.full((n_pad,), float(S), dtype=np.float32)
    idxp[:n] = np.asarray(indices, dtype=np.float32)
    w1t = np.ascontiguousarray(np.asarray(W1, np.float32).T)   # [C, HID]
    w2t = np.ascontiguousarray(np.asarray(W2, np.float32).T)   # [HID, C]
    iota_row = np.tile(np.arange(S, dtype=np.float32), (P, 1))
    iota_col = (np.arange(P, dtype=np.float32) % 32).reshape(P, 1)
    xs = xp.reshape(N_CORES, rows_per_core, C)
    xhs = xh.reshape(N_CORES, rows_per_core, C)
    idxs = idxp.reshape(N_CORES, rows_per_core)
    return [
        {
            "x": xs[c],
            "xh": xhs[c],
            "idxf": idxs[c],
            "idx8": idxs[c].astype(NP_FP8),
            "idxp": _permute_idx_p1(idxs[c], subtiles, t_chunk).astype(NP_FP8),
            "w1t": w1t,
            "w2t": w2t,
            "iota_row": iota_row,
            "iota_col": iota_col,
        }
        for c in range(N_CORES)
    ]


def kernel(x, indices, W1, W2, _trace=False, _trace_kwargs=None):
    n = x.shape[0]
    nc = _get_nc()
    in_maps = make_in_maps(x, indices, W1, W2)
    res = run_bass_kernel_spmd(
        nc, in_maps, core_ids=list(range(N_CORES)), trace=_trace,
        **(_trace_kwargs or {}),
    )
    out = np.concatenate([res.results[c]["out"] for c in range(N_CORES)],
                         axis=0)[:n]
    if _trace:
        return out, res
    return out



# revision 10
# speedup vs baseline: 1.8579x; 1.8579x over previous
"""Trainium2 Bass kernel for FlattenSELayer (segment mean -> SE MLP -> gather
multiply), data-parallel over 8 NeuronCores.

Per core (rows sharded across cores):
  pass 1: segment-sum via PE matmuls with fp8 x sub-tiles stationary (FWL) and
          per-row one-hot(idx) as the moving operand; counts via ones-vector
          matmuls. AllGather of the tiny (129,16) partial over the 8 cores
          (a dummy 1x8 AllGather issued at t=0 absorbs the first-collective
          barrier/init cost concurrently with pass 1).
  epilogue: pooled = seg_sum/counts, SE MLP (relu/sigmoid) -> gate (16,128)
          downcast to bf16.
  pass 2: channel-major. gate bf16 [16,128] is the single stationary operand;
          a host-staged one-hot [16,R] bf16 streams as the moving operand in
          N=512 column blocks -> PSUM holds gate[idx[n],c] as [128,512].
          DVE multiplies with the host-staged x^T bf16 [128,R]; out written
          bf16 [128,R] and transposed back on host.

Traffic per core ~84 MB (16 fp8 + 32 bf16 + 4 oh reads, 32 bf16 write) vs
~148 MB for the two-pass f32 variant; PE time ~0.1 ms -> ~0.1x of DMA.
"""
import sys
import types

import numpy as np

# ── shim the missing antenv.axon_hooks so run_bass_kernel_spmd imports ──
if "antenv.axon_hooks" not in sys.modules:
    _hooks = types.ModuleType("antenv.axon_hooks")
    _hooks._hook = None
    _hooks.set_axon_ntff_profile_hook = lambda h: setattr(_hooks, "_hook", h)
    _hooks.get_axon_ntff_profile_hook = lambda: _hooks._hook
    sys.modules["antenv.axon_hooks"] = _hooks
    import antenv

    antenv.axon_hooks = _hooks

import concourse.bass as bass
import concourse.bacc as bacc
import concourse.tile as tile
import concourse.mybir as mybir
from concourse.bass_utils import run_bass_kernel_spmd

F32 = mybir.dt.float32
BF16 = mybir.dt.bfloat16
FP8 = mybir.dt.float8e4
NP_BF16 = mybir.dt.np(BF16)
NP_FP8 = mybir.dt.np(FP8)

N_CORES = 8
P = 128          # partitions / rows per pass-1 sub-tile
C = 128          # channels
S = 16           # num segments
HID = 32         # SE hidden dim

N_FULL = 1_000_000
TILE2 = 512      # pass-2 rows per matmul (one PSUM bank)
CHUNK2 = 8       # pass-2 tiles per DMA chunk (4096 rows)
T1_CHUNK = 64    # pass-1 sub-tiles per DMA chunk

ROWS_PER_CORE = 125440                   # divisible by 128 and 512
SUBTILES = ROWS_PER_CORE // P            # 980
N_PAD = ROWS_PER_CORE * N_CORES          # 1003520


def _chunks(total, step):
    out = []
    done = 0
    while done < total:
        t = min(step, total - done)
        out.append((done, t))
        done += t
    return out


def build_kernel(rows_per_core=ROWS_PER_CORE):
    assert rows_per_core % P == 0 and rows_per_core % TILE2 == 0
    subtiles = rows_per_core // P
    chunks1 = [(b * P, t) for b, t in _chunks(subtiles, T1_CHUNK)]
    ntiles2 = rows_per_core // TILE2
    chunks2 = _chunks(ntiles2, CHUNK2)

    nc = bacc.Bacc("TRN2", target_bir_lowering=False, debug=False,
                   num_devices=N_CORES)

    # x twice: fp8 row-major for pass-1 segment sums, bf16 channel-major for
    # pass 2's gather-multiply
    xh_in = nc.dram_tensor("xh", [rows_per_core, C], FP8,
                           kind="ExternalInput")
    xt_in = nc.dram_tensor("xt", [C, rows_per_core], BF16,
                           kind="ExternalInput")
    # pass-2 one-hot, host-built: rows 0-15 and 16-31 both hold
    # (idx == s) so a single matmul applies the fp8 hi+lo gate split
    oh_in = nc.dram_tensor("oh", [2 * S, rows_per_core], FP8,
                           kind="ExternalInput")
    # pass-1 per-partition idx, host-permuted: [128, subtiles] where column
    # block u holds idx[base_u + p*tu + t]
    idxp_in = nc.dram_tensor("idxp", [P, subtiles], FP8,
                             kind="ExternalInput")
    w1t_in = nc.dram_tensor("w1t", [C, HID], F32, kind="ExternalInput")
    w2t_in = nc.dram_tensor("w2t", [HID, C], F32, kind="ExternalInput")
    iota_row_in = nc.dram_tensor("iota_row", [P, S], F32,
                                 kind="ExternalInput")
    out_t = nc.dram_tensor("out", [C, rows_per_core], BF16,
                           kind="ExternalOutput")

    xh_ap = xh_in.ap()
    xt_ap = xt_in.ap()
    oh_ap = oh_in.ap()
    out_ap = out_t.ap()

    with tile.TileContext(nc) as tc:
        with (
            tc.tile_pool(name="cst", bufs=1) as cst,
            tc.tile_pool(name="xp1", bufs=3) as xp1,
            tc.tile_pool(name="oh1", bufs=3) as oh1,
            tc.tile_pool(name="xp2", bufs=12) as xp2,
            tc.tile_pool(name="ohp", bufs=6) as ohp,
            tc.tile_pool(name="op2", bufs=3) as op2,
            tc.tile_pool(name="dram", bufs=1, space="DRAM") as dram,
        ):
            # ── dummy collective at t=0: absorbs first-cc barrier/init ──
            z8 = cst.tile([1, 8], F32)
            nc.vector.memset(z8[:], 0.0)
            warm_in = dram.tile([1, 8], F32)
            nc.gpsimd.dma_start(out=warm_in[:], in_=z8[:])
            warm_out = dram.tile([N_CORES, 1, 8], F32, addr_space="Shared")
            nc.gpsimd.collective_compute(
                "AllGather",
                mybir.AluOpType.bypass,
                replica_groups=[list(range(N_CORES))],
                ins=[warm_in[:].opt()],
                outs=[warm_out[:].opt()],
            )

            # constants
            iota_row = cst.tile([P, S], F32)
            nc.sync.dma_start(out=iota_row[:], in_=iota_row_in.ap())
            idx_p1 = cst.tile([P, subtiles], FP8)
            nc.sync.dma_start(out=idx_p1[:], in_=idxp_in.ap())
            w1t_sb = cst.tile([C, HID], F32)
            nc.sync.dma_start(out=w1t_sb[:], in_=w1t_in.ap())
            w2t_sb = cst.tile([HID, C], F32)
            nc.sync.dma_start(out=w2t_sb[:], in_=w2t_in.ap())
            ones128 = cst.tile([P, 1], FP8)
            nc.vector.memset(ones128[:], 1.0)
            ones_row = cst.tile([1, P], F32)
            nc.vector.memset(ones_row[:], 1.0)

            # ───────────────────────── pass 1 ─────────────────────────
            with tc.tile_pool(name="ps1", bufs=1, space="PSUM") as ps1:
                psum_seg = ps1.tile([C, S], F32)
                psum_cnt = ps1.tile([1, TILE2], F32)

                n_cnt = 0
                n_cnt_total = sum(len(_chunks(tu, 32)) for _, tu in chunks1)
                n_sub_done = 0
                sub_off = 0
                for base, tu in chunks1:
                    rows = tu * P
                    x_t = xp1.tile([P, tu, C], FP8, tag="x1", name="x1")
                    nc.sync.dma_start(
                        out=x_t[:],
                        in_=xh_ap[base:base + rows].rearrange(
                            "(p t) c -> p t c", p=P, t=tu),
                    )
                    idx_t = idx_p1[:, sub_off:sub_off + tu]
                    sub_off += tu
                    oh_t = oh1.tile([P, tu, S], FP8, tag="oh1", name="oh1")
                    idx_b = bass.AP(tensor=idx_t.tensor,
                                    offset=idx_t.offset,
                                    ap=[idx_t.ap[0], idx_t.ap[1], [0, S]])
                    iota_b = bass.AP(tensor=iota_row[:].tensor,
                                     offset=iota_row[:].offset,
                                     ap=[iota_row[:].ap[0], [0, tu],
                                         iota_row[:].ap[1]])
                    nc.vector.tensor_tensor(oh_t[:], idx_b, iota_b,
                                            mybir.AluOpType.is_equal)
                    for c0, ct in _chunks(tu, 32):
                        n_cnt += 1
                        nc.tensor.matmul(
                            psum_cnt[:, 0:ct * S],
                            ones128[:],
                            oh_t[:, c0:c0 + ct, :].rearrange(
                                "p t s -> p (t s)"),
                            start=(n_cnt == 1),
                            stop=(n_cnt == n_cnt_total),
                        )
                    for t in range(tu):
                        n_sub_done += 1
                        nc.tensor.matmul(
                            psum_seg[:],
                            x_t[:, t, :],
                            oh_t[:, t, :],
                            start=(n_sub_done == 1),
                            stop=(n_sub_done == subtiles),
                        )

                # ─────────────────── epilogue / MLP ───────────────────
                seg_sb = cst.tile([C, S], F32)
                nc.vector.tensor_copy(seg_sb[:], psum_seg[:])
                cnt_sb = cst.tile([1, TILE2], F32)
                nc.vector.tensor_copy(cnt_sb[:], psum_cnt[:])
                w = TILE2
                while w > S:
                    w //= 2
                    nc.vector.tensor_tensor(cnt_sb[:, 0:w], cnt_sb[:, 0:w],
                                            cnt_sb[:, w:2 * w],
                                            mybir.AluOpType.add)
                cnt16 = cnt_sb[:, 0:S]

                bounce_in = dram.tile([P + 1, S], F32)
                nc.scalar.dma_start(out=bounce_in[0:C, :], in_=seg_sb[:])
                nc.scalar.dma_start(out=bounce_in[C:C + 1, :], in_=cnt16)
                bounce_out = dram.tile([N_CORES, P + 1, S], F32,
                                       addr_space="Shared")
                nc.gpsimd.collective_compute(
                    "AllGather",
                    mybir.AluOpType.bypass,
                    replica_groups=[list(range(N_CORES))],
                    ins=[bounce_in[:].opt()],
                    outs=[bounce_out[:].opt()],
                )
                bo = bounce_out[:]
                seg_r = cst.tile([C, N_CORES, S], F32)
                nc.scalar.dma_start(
                    out=seg_r[:],
                    in_=bass.AP(tensor=bo.tensor, offset=bo.offset,
                                ap=[[S, C], [(P + 1) * S, N_CORES],
                                    [1, S]]),
                )
                cnt_r = cst.tile([1, N_CORES, S], F32)
                nc.scalar.dma_start(
                    out=cnt_r[:],
                    in_=bass.AP(tensor=bo.tensor,
                                offset=bo.offset + C * S,
                                ap=[[0, 1], [(P + 1) * S, N_CORES],
                                    [1, S]]),
                )
                w = N_CORES
                while w > 1:
                    w //= 2
                    nc.vector.tensor_tensor(
                        seg_r[:, 0:w, :], seg_r[:, 0:w, :],
                        seg_r[:, w:2 * w, :], mybir.AluOpType.add)
                    nc.vector.tensor_tensor(
                        cnt_r[:, 0:w, :], cnt_r[:, 0:w, :],
                        cnt_r[:, w:2 * w, :], mybir.AluOpType.add)
                seg_g = seg_r[:, 0, :]
                cnt_g = cnt_r[:, 0, :]

                nc.vector.tensor_scalar(cnt_g, cnt_g, 1.0, None,
                                        mybir.AluOpType.max)
                rcnt = cst.tile([1, S], F32)
                nc.vector.reciprocal(rcnt[:], cnt_g)
                rcnt_psum = ps1.tile([C, S], F32)
                nc.tensor.matmul(rcnt_psum[:], ones_row[:], rcnt[:],
                                 start=True, stop=True)
                pooledT = cst.tile([C, S], F32)
                nc.vector.tensor_tensor(pooledT[:], seg_g, rcnt_psum[:],
                                        mybir.AluOpType.mult)

                h_psum = ps1.tile([HID, S], F32)
                nc.tensor.matmul(h_psum[:], w1t_sb[:], pooledT[:],
                                 start=True, stop=True)
                hT_sb = cst.tile([HID, S], F32)
                nc.scalar.activation(hT_sb[:], h_psum[:],
                                     mybir.ActivationFunctionType.Relu)
                g_psum = ps1.tile([S, C], F32)
                nc.tensor.matmul(g_psum[:], hT_sb[:], w2t_sb[:],
                                 start=True, stop=True)
                gate_sb = cst.tile([S, C], F32)
                nc.scalar.activation(gate_sb[:], g_psum[:],
                                     mybir.ActivationFunctionType.Sigmoid)
                # split gate into fp8 hi + lo halves stacked [32, C]; the
                # doubled one-hot applies both in one matmul with ~bf16
                # accuracy at fp8 operand cost
                g32 = cst.tile([2 * S, C], FP8)
                nc.vector.tensor_copy(g32[0:S, :], gate_sb[:])
                g_lo = cst.tile([S, C], FP8)
                nc.vector.tensor_tensor(g_lo[:], gate_sb[:], g32[0:S, :],
                                        mybir.AluOpType.subtract)
                # engine writes must start at a 32-aligned partition, so
                # place the lo half at partitions 16-31 with a DMA copy
                nc.scalar.dma_start(out=g32[S:2 * S, :], in_=g_lo[:])

            # ───────────────────────── pass 2 ─────────────────────────
            with tc.tile_pool(name="ps2", bufs=3, space="PSUM") as ps2:
                for t0, nt in chunks2:
                    b0 = t0 * TILE2
                    cols = nt * TILE2
                    xt_t = xp2.tile([C, CHUNK2 * TILE2], BF16, tag="x2",
                                    name="x2")
                    nc.sync.dma_start(
                        out=xt_t[:, 0:cols],
                        in_=bass.AP(tensor=xt_ap.tensor,
                                    offset=xt_ap.offset + b0,
                                    ap=[[rows_per_core, C], [1, cols]]),
                    )
                    oh_t = ohp.tile([2 * S, CHUNK2 * TILE2], FP8, tag="oh2",
                                    name="oh2")
                    nc.gpsimd.dma_start(
                        out=oh_t[:, 0:cols],
                        in_=bass.AP(tensor=oh_ap.tensor,
                                    offset=oh_ap.offset + b0,
                                    ap=[[rows_per_core, 2 * S], [1, cols]]),
                    )
                    o_t = op2.tile([C, CHUNK2 * TILE2], BF16, tag="o2",
                                   name="o2")
                    for j0 in range(0, nt, 2):
                        pr = min(2, nt - j0)
                        w = pr * TILE2
                        g_ps = ps2.tile([C, 2, TILE2], F32, tag="g",
                                        name="g")
                        for j in range(pr):
                            nc.tensor.matmul(
                                g_ps[:, j, :],
                                g32[:],
                                oh_t[:, (j0 + j) * TILE2:
                                     (j0 + j + 1) * TILE2],
                                start=True, stop=True,
                            )
                        nc.vector.tensor_tensor(
                            o_t[:, j0 * TILE2:j0 * TILE2 + w],
                            xt_t[:, j0 * TILE2:j0 * TILE2 + w],
                            g_ps[:, 0:pr, :].rearrange("p a b -> p (a b)"),
                            mybir.AluOpType.mult,
                        )
                    nc.scalar.dma_start(
                        out=bass.AP(tensor=out_ap.tensor,
                                    offset=out_ap.offset + b0,
                                    ap=[[rows_per_core, C], [1, cols]]),
                        in_=o_t[:, 0:cols],
                    )

    nc.compile()
    return nc


_NC_CACHE = {}


def _get_nc(rows_per_core=ROWS_PER_CORE):
    if rows_per_core not in _NC_CACHE:
        _NC_CACHE[rows_per_core] = build_kernel(rows_per_core)
    return _NC_CACHE[rows_per_core]


def _permute_idx_p1(idx_core, subtiles):
    """[rows] -> [128, subtiles]; block u holds idx[base_u + p*tu + t]."""
    cols = []
    for b, tu in _chunks(subtiles, T1_CHUNK):
        cols.append(idx_core[b * P:(b + tu) * P].reshape(P, tu))
    return np.concatenate(cols, axis=1)


def make_in_maps(x, indices, W1, W2, rows_per_core=ROWS_PER_CORE):
    n = x.shape[0]
    subtiles = rows_per_core // P
    n_pad = rows_per_core * N_CORES
    xp = np.zeros((n_pad, C), dtype=np.float32)
    xp[:n] = np.asarray(x, dtype=np.float32)
    xh = xp.astype(NP_FP8)
    idxp = np.full((n_pad,), float(S), dtype=np.float32)
    idxp[:n] = np.asarray(indices, dtype=np.float32)
    w1t = np.ascontiguousarray(np.asarray(W1, np.float32).T)   # [C, HID]
    w2t = np.ascontiguousarray(np.asarray(W2, np.float32).T)   # [HID, C]
    iota_row = np.tile(np.arange(S, dtype=np.float32), (P, 1))
    seg_iota = np.arange(S, dtype=np.float32)[:, None]
    xs = xp.reshape(N_CORES, rows_per_core, C)
    xhs = xh.reshape(N_CORES, rows_per_core, C)
    idxs = idxp.reshape(N_CORES, rows_per_core)
    return [
        {
            "xh": xhs[c],
            "xt": np.ascontiguousarray(xs[c].T).astype(NP_BF16),
            "oh": np.tile((idxs[c][None, :] == seg_iota), (2, 1)
                          ).astype(NP_FP8),
            "idxp": _permute_idx_p1(idxs[c], subtiles).astype(NP_FP8),
            "w1t": w1t,
            "w2t": w2t,
            "iota_row": iota_row,
        }
        for c in range(N_CORES)
    ]


def kernel(x, indices, W1, W2, _trace=False, _trace_kwargs=None):
    n = x.shape[0]
    nc = _get_nc()
    in_maps = make_in_maps(x, indices, W1, W2)
    res = run_bass_kernel_spmd(
        nc, in_maps, core_ids=list(range(N_CORES)), trace=_trace,
        **(_trace_kwargs or {}),
    )
    out = np.concatenate(
        [res.results[c]["out"].astype(np.float32).T for c in range(N_CORES)],
        axis=0)[:n]
    if _trace:
        return out, res
    return out
